# revision 1
# baseline (speedup 1.0000x reference)
"""Multi-head causal attention (B=2, T=4096, C=768, H=12) on 8 TRN2 NeuronCores.

Sharding: 24 (batch, head) units -> 3 heads per core; cores 0-3 take batch 0,
cores 4-7 batch 1. Each core computes Q/K/V projections for its 3 heads, full-T
causal attention, and a partial output projection [C, T]. Host sums the 4
partials per batch and adds the output bias.

Device layouts (per core):
  xT   [768, T] bf16   x[b] transposed (c-major) - input
  Q, K [d, t] bf16     head-pair tiles [128, 512] (partitions = 2x64 head dims);
                       head2 in separate [64, 512] tiles at base partition 0
  V    [t, d] bf16     per 128-row tile [128, 3*65] (65th col = ones -> denom)
  att^T [k, q]         QK^T computed transposed (lhsT=K-tile, rhs=Q-tile);
                       heads 0/1 interleaved -> concurrent PE row groups
  exp   bf16           ACT Exp from PSUM, causal mask applied as 0/1 multiply
  y^T  [65, 512] psum  accum over k-tiles (row 64 = softmax denominator)
  out  [768, T] f32    partial out-projection, c_out-major
"""

import ml_dtypes
import numpy as np

import concourse.bass as bass
import concourse.tile as tile
from concourse import bacc, mybir

F32 = mybir.dt.float32
BF16 = mybir.dt.bfloat16
AF = mybir.ActivationFunctionType

N_CORES = 8
T = 4096
C = 768
H = 12
D = 64
HPC = 3          # heads per core
QT = 512         # q-tile width (matmul N)
KT = 128         # k-tile width (partition dim)
NCH = C // 128   # 6 contraction chunks over C


def build_nc(t=T):
    nt = t // QT          # q/t tiles of 512
    nsub = t // KT        # t sub-tiles of 128

    nc = bacc.Bacc("TRN2", target_bir_lowering=False, debug=False)

    xT = nc.declare_dram_parameter("xT", [C, t], BF16, isOutput=False)
    wqk = nc.declare_dram_parameter("wqk", [C, 384], BF16, isOutput=False)
    bqk = nc.declare_dram_parameter("bqk", [128, 4], F32, isOutput=False)
    wv = nc.declare_dram_parameter("wv", [C, 192], BF16, isOutput=False)
    bv = nc.declare_dram_parameter("bv", [64, 3], F32, isOutput=False)
    wo = nc.declare_dram_parameter("wo", [64, 3 * C], BF16, isOutput=False)
    msk = nc.declare_dram_parameter("msk", [128, 4 * QT], BF16, isOutput=False)
    out = nc.declare_dram_parameter("out", [C, t], F32, isOutput=True)

    xT_r = xT.ap().rearrange("(a p) t -> p a t", p=128)
    wqk_r = wqk.ap().rearrange("(a p) m -> p a m", p=128)
    wv_r = wv.ap().rearrange("(a p) m -> p a m", p=128)

    with tile.TileContext(nc) as tc:
        with (
            tc.tile_pool(name="const", bufs=1) as const_pool,
            tc.tile_pool(name="xt", bufs=4) as xp,
            tc.tile_pool(name="qp", bufs=nt) as qp,
            tc.tile_pool(name="kp", bufs=nt) as kp,
            tc.tile_pool(name="q2p", bufs=nt) as q2p,
            tc.tile_pool(name="k2p", bufs=nt) as k2p,
            tc.tile_pool(name="vp", bufs=nsub) as vp,
            tc.tile_pool(name="yp", bufs=HPC * nt) as yp,
            tc.tile_pool(name="ep", bufs=8) as ep,
            tc.tile_pool(name="op", bufs=4) as op,
            tc.tile_pool(name="sp", bufs=6) as sp,
            tc.tile_pool(name="ps_main", bufs=3, space="PSUM") as ps_main,
            tc.tile_pool(name="ps_y", bufs=2, space="PSUM") as ps_y,
        ):
            # ---- constants ----
            wqk_sb = const_pool.tile([128, NCH, 384], BF16, tag="wqk")
            nc.sync.dma_start(out=wqk_sb, in_=wqk_r)
            wv_sb = const_pool.tile([128, NCH, 192], BF16, tag="wv")
            nc.sync.dma_start(out=wv_sb, in_=wv_r)
            wo_sb = const_pool.tile([64, 3 * C], BF16, tag="wo")
            nc.sync.dma_start(out=wo_sb, in_=wo.ap())
            bqk_sb = const_pool.tile([128, 4], F32, tag="bqk")
            nc.sync.dma_start(out=bqk_sb, in_=bqk.ap())
            bv_sb = const_pool.tile([64, 3], F32, tag="bv")
            nc.sync.dma_start(out=bv_sb, in_=bv.ap())
            mask_sb = const_pool.tile([128, 4, QT], BF16, tag="msk")
            msk_r = msk.ap().rearrange("p (o q) -> p o q", q=QT)
            nc.sync.dma_start(out=mask_sb, in_=msk_r)

            q_t, k_t, q2_t, k2_t, v_t = [], [], [], [], []
            y_t = [[None] * nt for _ in range(HPC)]

            # M-tile column ranges in wqk: [Qh0|Qh1](128), [Kh0|Kh1](128),
            # [Qh2](64), [Kh2](64) - all outputs at base partition 0.
            m_cols = [(0, 128), (128, 256), (256, 320), (320, 384)]

            # ---- phase A: projections ----
            for ti in range(nt):
                xt = xp.tile([128, NCH, QT], BF16, tag="xt")
                nc.sync.dma_start(out=xt, in_=xT_r[:, :, ti * QT:(ti + 1) * QT])
                dsts = []
                for m in range(4):
                    c0, c1 = m_cols[m]
                    mp = c1 - c0
                    ps = ps_main.tile([128, QT], F32, tag="ps")
                    for ci in range(NCH):
                        nc.tensor.matmul(
                            ps[0:mp, :],
                            lhsT=wqk_sb[:, ci, c0:c1],
                            rhs=xt[:, ci, :],
                            start=(ci == 0),
                            stop=(ci == NCH - 1),
                        )
                    pool = (qp, kp, q2p, k2p)[m]
                    dst = pool.tile([mp, QT], BF16, tag=("q", "k", "q2", "k2")[m])
                    nc.vector.tensor_scalar_add(dst, ps[0:mp, :], bqk_sb[0:mp, m:m + 1])
                    dsts.append(dst)
                q_t.append(dsts[0])
                k_t.append(dsts[1])
                q2_t.append(dsts[2])
                k2_t.append(dsts[3])
                for si in range(4):
                    psv = ps_main.tile([128, 192], F32, tag="ps")
                    for ci in range(NCH):
                        nc.tensor.matmul(
                            psv,
                            lhsT=xt[:, ci, si * 128:(si + 1) * 128],
                            rhs=wv_sb[:, ci, :],
                            start=(ci == 0),
                            stop=(ci == NCH - 1),
                        )
                    vt = vp.tile([128, HPC * 65], BF16, tag="v")
                    vt_r = vt.rearrange("p (h e) -> p h e", e=65)
                    nc.vector.memset(vt_r[:, :, 64:65], 1.0)
                    nc.vector.tensor_copy(
                        vt_r[:, :, 0:64],
                        psv[:, 0:HPC * 64].rearrange("p (h e) -> p h e", e=64),
                    )
                    v_t.append(vt)

            def normalize(h, qi, y_ps):
                # y_ps row 64 = denominator (PSUM partition 64). DVE-copy it
                # to SBUF partition 64, broadcast to partitions 0-63 on the
                # (otherwise idle) GpSimd engine, reciprocal there.
                den64 = sp.tile([65, QT], F32, tag="den64")
                nc.vector.tensor_copy(den64[64:65, :], y_ps[64:65, :])
                den = sp.tile([1, QT], F32, tag="den")
                nc.sync.dma_start(out=den, in_=den64[64:65, :])
                bc_sb = sp.tile([64, QT], F32, tag="bcs")
                nc.gpsimd.partition_broadcast(bc_sb, den[0:1, :])
                rec = sp.tile([64, QT], F32, tag="rec")
                nc.vector.reciprocal_approx_fast(rec, bc_sb)
                yt = yp.tile([64, QT], BF16, tag="y")
                nc.vector.tensor_mul(yt, y_ps[0:64, :], rec)
                nc.vector.tensor_scalar_add(yt, yt, bv_sb[:, h:h + 1])
                y_t[h][qi] = yt

            # ---- phase B + C, interleaved per q-tile ----
            # Heads 0 and 1 interleaved per k-tile: their QK matmuls use PE
            # row groups 0-1 and 2-3 (lhsT base partitions 0 / 64) and run
            # concurrently; one Exp covers both heads' tiles.
            for qi in range(nt):
                q0_ap = q_t[qi][0:64, :]
                q1_ap = q_t[qi][64:128, :]
                y0 = ps_y.tile([65, QT], F32, tag="psy")
                y1 = ps_y.tile([65, QT], F32, tag="psy")
                n_k = 4 * qi + 4
                for kt in range(n_k):
                    tj, tcol = kt // 4, (kt % 4) * 128
                    aps = ps_main.tile([128, 2, QT], F32, tag="ps")
                    nc.tensor.matmul(
                        aps[:, 0, :], lhsT=k_t[tj][0:64, tcol:tcol + 128],
                        rhs=q0_ap, start=True, stop=True,
                    )
                    nc.tensor.matmul(
                        aps[:, 1, :], lhsT=k_t[tj][64:128, tcol:tcol + 128],
                        rhs=q1_ap, start=True, stop=True,
                    )
                    et = ep.tile([128, 2, QT], BF16, tag="e")
                    nc.scalar.activation(et, aps, AF.Exp)
                    o = kt - 4 * qi
                    if o >= 0:
                        nc.vector.tensor_mul(
                            et, et, mask_sb[:, o:o + 1, :].to_broadcast([128, 2, QT])
                        )
                    for u, yps in ((0, y0), (1, y1)):
                        nc.tensor.matmul(
                            yps,
                            lhsT=v_t[kt][:, u * 65:(u + 1) * 65],
                            rhs=et[:, u, :],
                            start=(kt == 0),
                            stop=(kt == n_k - 1),
                        )
                normalize(0, qi, y0)
                normalize(1, qi, y1)

                # head 2 for this q-tile: pairs of k-tiles per Exp
                q_ap = q2_t[qi][:, :]
                y2 = ps_y.tile([65, QT], F32, tag="psy")
                for pj in range(n_k // 2):
                    aps = ps_main.tile([128, 2, QT], F32, tag="ps")
                    for u in (0, 1):
                        kt = 2 * pj + u
                        tj, tcol = kt // 4, (kt % 4) * 128
                        nc.tensor.matmul(
                            aps[:, u, :], lhsT=k2_t[tj][:, tcol:tcol + 128],
                            rhs=q_ap, start=True, stop=True,
                        )
                    et = ep.tile([128, 2, QT], BF16, tag="e")
                    nc.scalar.activation(et, aps, AF.Exp)
                    for u in (0, 1):
                        o = 2 * pj + u - 4 * qi
                        if o >= 0:
                            nc.vector.tensor_mul(
                                et[:, u, :], et[:, u, :], mask_sb[:, o, :]
                            )
                    for u in (0, 1):
                        kt = 2 * pj + u
                        nc.tensor.matmul(
                            y2,
                            lhsT=v_t[kt][:, 2 * 65:3 * 65],
                            rhs=et[:, u, :],
                            start=(kt == 0),
                            stop=(kt == n_k - 1),
                        )
                normalize(2, qi, y2)

                # out-projection partial for this q-tile
                for mo in range(NCH):
                    ps = ps_main.tile([128, QT], F32, tag="ps")
                    for h in range(HPC):
                        nc.tensor.matmul(
                            ps,
                            lhsT=wo_sb[:, h * C + mo * 128:h * C + (mo + 1) * 128],
                            rhs=y_t[h][qi],
                            start=(h == 0),
                            stop=(h == HPC - 1),
                        )
                    ot = op.tile([128, QT], F32, tag="o")
                    nc.vector.tensor_copy(ot, ps)
                    nc.sync.dma_start(
                        out=out.ap()[mo * 128:(mo + 1) * 128,
                                     qi * QT:(qi + 1) * QT],
                        in_=ot,
                    )

    nc.compile()
    return nc


def make_mask():
    i = np.arange(128)[:, None]
    j = np.arange(QT)[None, :]
    m = np.zeros((128, 4 * QT), np.float32)
    for o in range(4):
        m[:, o * QT:(o + 1) * QT] = (j >= o * 128 + i)
    return m


def shard_inputs(x, Wq, bq, Wk, bk, Wv, bv, Wo, bo, t=T):
    """Build per-core in_maps."""
    s = 1.0 / np.sqrt(D)
    mask = make_mask()
    bf = ml_dtypes.bfloat16
    in_maps = []
    for c in range(N_CORES):
        b = c // (N_CORES // x.shape[0])
        h0 = HPC * (c % 4)
        hs = slice(h0 * D, (h0 + HPC) * D)
        Wq_s = (Wq[hs] * s).astype(np.float32)
        bq_s = (bq[hs] * s).astype(np.float32)
        Wk_s, bk_s = Wk[hs], bk[hs]
        wqk = np.concatenate(
            [Wq_s[0:128].T, Wk_s[0:128].T, Wq_s[128:192].T, Wk_s[128:192].T], axis=1
        )  # [768, 384]
        bqk = np.zeros((128, 4), np.float32)
        bqk[:, 0] = bq_s[0:128]
        bqk[:, 1] = bk_s[0:128]
        bqk[0:64, 2] = bq_s[128:192]
        bqk[0:64, 3] = bk_s[128:192]
        wv = np.ascontiguousarray(Wv[hs].T.astype(np.float32))
        bv_s = bv[hs].reshape(HPC, D).T  # [64, 3]
        wo = np.concatenate(
            [Wo[:, hs][:, h * D:(h + 1) * D].T for h in range(HPC)], axis=1
        )  # [64, 3*768]
        in_maps.append({
            "xT": np.ascontiguousarray(x[b].T).astype(bf),
            "wqk": np.ascontiguousarray(wqk).astype(bf),
            "bqk": np.ascontiguousarray(bqk),
            "wv": wv.astype(bf),
            "bv": np.ascontiguousarray(bv_s),
            "wo": np.ascontiguousarray(wo).astype(bf),
            "msk": mask.astype(bf),
        })
    return in_maps


_NC_CACHE = {}


def get_nc(t=T):
    if t not in _NC_CACHE:
        _NC_CACHE[t] = build_nc(t)
    return _NC_CACHE[t]


def run_cores(in_maps, t=T, trace=False, tmpdir=None):
    from concourse.bass_utils import run_bass_kernel_spmd

    nc = get_nc(t)
    return run_bass_kernel_spmd(
        nc, in_maps, list(range(N_CORES)), trace=trace, tmpdir=tmpdir
    )


def gather(results, x_shape, bo):
    B, t, _ = x_shape
    out = np.zeros((B, t, C), np.float32)
    for c in range(N_CORES):
        b = c // (N_CORES // B)
        out[b] += results[c]["out"].T
    out += bo[None, None, :]
    return out


def kernel(x, Wq, bq, Wk, bk, Wv, bv, Wo, bo, _trace=False, _tmpdir=None):
    x = np.asarray(x, dtype=np.float32)
    args = [np.asarray(a, dtype=np.float32) for a in (Wq, bq, Wk, bk, Wv, bv, Wo, bo)]
    Wq, bq, Wk, bk, Wv, bv, Wo, bo = args
    t = x.shape[1]
    in_maps = shard_inputs(x, Wq, bq, Wk, bk, Wv, bv, Wo, bo, t=t)
    res = run_cores(in_maps, t=t, trace=_trace, tmpdir=_tmpdir)
    out = gather(res.results, x.shape, bo)
    kernel.last_result = res
    return out

